# revision 1
# baseline (speedup 1.0000x reference)
"""Multi-head attention (BaselineAttention) Bass kernel for 8 trn2 NeuronCores.

Problem: x[4,2048,1024], per-head Wq/Wk/Wv [16,1024,64] (+biases), Wo[1024,1024]+bo.
Sharding: core c -> batch b=c//2, head-group g=c%2 (8 heads each).
Each core computes y_partial[b] = sum_{h in group} softmax(qk^T/8) v @ Wo_rows(h).
Host combines: y[b] = part[2b] + part[2b+1] + bo + bv@Wo  (bv folded out of device).

Device algorithm per core (all matmuls fp32r = full-speed ~tf32 precision):
  xT[d,s] resident; qT/kT[(h,e),s] via W^T@xT; v[t,(h,e)] via xT^T@Wv.
  Per head-pair, per s-chunk(512): scores^T[t,s] = kT^T qT (K=64, row-packed
  pairs); exp via ACT (scale=1/8) in [128,1024] tiles; o~aug^T = [v|1]^T @
  exp (M=65, psum-accumulated over t-tiles); r=row 64; normalization via
  gpsimd partition-broadcast of r + DVE reciprocal/multiply.
  Out-proj: y[s,:] accumulated over the 4 pair K-blocks with Wo rows.
"""
import numpy as np

B, S, DIM, H, DH = 4, 2048, 1024, 16, 64
NCORES = 8
HPC = H // 2          # heads per core = 8
NPAIR = HPC // 2      # head pairs per core = 4
NT = S // 128         # t-tiles = 16
NSQ = S // 512        # s-chunks of 512 = 4
NKT = DIM // 128      # d-tiles = 8
SCALE = 1.0 / float(np.sqrt(DH))

_CACHE = {}


def _build(S=S, DIM=DIM, NPAIR=NPAIR, ncores=NCORES, debug_taps=False, repeat=1,
           skip_qkv=False, skip_attn=False, skip_oproj=False):
    HPC = 2 * NPAIR
    NT = S // 128
    NSQ = S // 512
    NKT = DIM // 128
    import concourse.bass as bass
    import concourse.mybir as mybir
    import concourse.tile as tile
    from concourse import bacc

    f32 = mybir.dt.float32
    f32r = mybir.dt.float32r
    AF = mybir.ActivationFunctionType
    Alu = mybir.AluOpType

    nc = bacc.Bacc("TRN2", target_bir_lowering=False, debug=False,
                   num_devices=ncores)

    xT_d = nc.dram_tensor("xT", [DIM, S], f32r, kind="ExternalInput")
    wq_d = nc.dram_tensor("wq", [DIM, HPC * DH], f32r, kind="ExternalInput")
    wk_d = nc.dram_tensor("wk", [DIM, HPC * DH], f32r, kind="ExternalInput")
    wv_d = nc.dram_tensor("wv", [DIM, HPC * DH], f32r, kind="ExternalInput")
    bq_d = nc.dram_tensor("bq", [128, NPAIR], f32, kind="ExternalInput")
    bk_d = nc.dram_tensor("bk", [128, NPAIR], f32, kind="ExternalInput")
    wo_d = nc.dram_tensor("wo", [HPC * DH, DIM], f32r, kind="ExternalInput")
    y_d = nc.dram_tensor("y", [S, DIM], f32, kind="ExternalOutput")
    if debug_taps:
        dbg_qT = nc.dram_tensor("dbg_qT", [128, NPAIR, S], f32,
                                kind="ExternalOutput")
        dbg_kT = nc.dram_tensor("dbg_kT", [128, NPAIR, S], f32,
                                kind="ExternalOutput")
        dbg_vA = nc.dram_tensor("dbg_vA", [128, HPC * NT * 65], f32,
                                kind="ExternalOutput")
        dbg_on = nc.dram_tensor("dbg_on", [128, NPAIR, S], f32,
                                kind="ExternalOutput")
        dbg_att = nc.dram_tensor("dbg_att", [128, 2, 2, 512], f32,
                                 kind="ExternalOutput")

    with tile.TileContext(nc) as tc:
        with tc.tile_pool(name="persist", bufs=1) as pp:
            # ---- persistent SBUF ----
            qT = pp.tile([128, NPAIR, S], f32r)       # [(pair-row), pair, s]
            kT = pp.tile([128, NPAIR, S], f32r)
            vA = pp.tile([128, HPC, NT, 65], f32r)    # [t%128, h, tt, e|1]
            bqs = pp.tile([128, NPAIR], f32)
            bks = pp.tile([128, NPAIR], f32)
            nc.sync.dma_start(out=bqs, in_=bq_d.ap())
            nc.sync.dma_start(out=bks, in_=bk_d.ap())
            ones_stage = pp.tile([128, HPC * NT], f32)
            nc.vector.memset(ones_stage, 1.0)
            nc.vector.tensor_copy(
                vA.rearrange("p h t e -> p (h t) e")[:, :, 64:65],
                ones_stage[:, :, None])

            for rep in range(repeat):
              # =============== Phase 1: QKV projections (streamed xT) ====
              # x chunks [128,512] stream from DRAM; q/k accumulate in 8
              # parallel 1-bank psums (kt-outer); v is a second stream pass.
              with tc.tile_pool(name=f"qkv{rep}", bufs=1) as qp, \
                   tc.tile_pool(name=f"psA{rep}", bufs=1, space="PSUM") as psA:
                  projs = [] if skip_qkv else [("q", wq_d, qT, bqs),
                                               ("k", wk_d, kT, bks)]
                  def _wdma(nm, wd, mt):
                      w = wtiles[(nm, mt)]
                      nc.sync.dma_start(
                          out=w,
                          in_=wd.ap().rearrange("(kt p) m -> p kt m", p=128)
                          [:, :, mt * 128:(mt + 1) * 128])

                  wtiles = {}
                  for nm, wd, dst, bias in projs:
                      for mt in range(NPAIR):
                          wtiles[(nm, mt)] = qp.tile(
                              [128, NKT, 128], f32r,
                              name=f"w_{nm}{mt}_{rep}", tag=f"w{nm}{mt}",
                              bufs=1)
                  xT_src = xT_d.ap().rearrange("(kt p) s -> p kt s", p=128)
                  for sc in range(NSQ):
                      pstiles = {}
                      for nm, wd, dst, bias in projs:
                          for mt in range(NPAIR):
                              i = (0 if nm == "q" else NPAIR) + mt
                              pstiles[(nm, mt)] = psA.tile(
                                  [128, 512], f32, name=f"ps{nm}{mt}{sc}_{rep}",
                                  tag=f"ps{i}", bufs=1)
                      if sc == 0 and projs:
                          # interleave q-weight DMAs with the first chunks;
                          # k weights stream during the q pass; sc0 runs the
                          # projections in two passes over retained chunks.
                          xcs = []
                          for kt in range(NKT):
                              if kt < NPAIR:
                                  _wdma("q", wq_d, kt)
                              xc = qp.tile([128, 512], f32r, tag="xc", bufs=8,
                                           name=f"xc0_{kt}_{rep}")
                              nc.sync.dma_start(
                                  out=xc, in_=xT_src[:, kt, 0:512])
                              xcs.append(xc)
                          for mt in range(NPAIR):
                              _wdma("k", wk_d, mt)
                          for nm, wd, dst, bias in projs:
                              for kt in range(NKT):
                                  for mt in range(NPAIR):
                                      nc.tensor.matmul(
                                          pstiles[(nm, mt)],
                                          wtiles[(nm, mt)][:, kt, :],
                                          xcs[kt],
                                          start=(kt == 0),
                                          stop=(kt == NKT - 1))
                      else:
                          for kt in range(NKT):
                              xc = qp.tile([128, 512], f32r, tag="xc", bufs=8,
                                           name=f"xc{sc}_{kt}_{rep}")
                              nc.sync.dma_start(
                                  out=xc,
                                  in_=xT_src[:, kt, sc * 512:(sc + 1) * 512])
                              for nm, wd, dst, bias in projs:
                                  for mt in range(NPAIR):
                                      nc.tensor.matmul(
                                          pstiles[(nm, mt)],
                                          wtiles[(nm, mt)][:, kt, :],
                                          xc,
                                          start=(kt == 0),
                                          stop=(kt == NKT - 1))
                      for nm, wd, dst, bias in projs:
                          for mt in range(NPAIR):
                              nc.scalar.activation(
                                  dst[:, mt, sc * 512:(sc + 1) * 512],
                                  pstiles[(nm, mt)], AF.Identity,
                                  bias=bias[:, mt:mt + 1], scale=1.0)

                  # ---- v: second streamed pass over xT ----
                  wv_sb = qp.tile([128, NKT, HPC * DH], f32r, name=f"wvsb_{rep}")
                  nc.sync.dma_start(
                      out=wv_sb,
                      in_=wv_d.ap().rearrange("(kt p) m -> p kt m", p=128))
                  for ttg in ([] if skip_qkv else range(NT // 4)):
                      psv = {}
                      for j4 in range(4):
                          psv[j4] = psA.tile([128, HPC * DH], f32,
                                             name=f"psv{ttg}_{j4}_{rep}",
                                             tag=f"ps{j4}", bufs=1)
                      for kt in range(NKT):
                          vc = qp.tile([128, 512], f32r, tag="xc", bufs=8,
                                       name=f"vc{ttg}_{kt}_{rep}")
                          nc.sync.dma_start(
                              out=vc,
                              in_=xT_src[:, kt,
                                         ttg * 512:(ttg + 1) * 512])
                          for j4 in range(4):
                              nc.tensor.matmul(
                                  psv[j4],
                                  vc[:, j4 * 128:(j4 + 1) * 128],
                                  wv_sb[:, kt, :],
                                  start=(kt == 0), stop=(kt == NKT - 1))
                      for j4 in range(4):
                          tt = ttg * 4 + j4
                          nc.vector.tensor_copy(vA[:, :, tt, 0:64], psv[j4])

              if debug_taps:
                  nc.sync.dma_start(out=dbg_qT.ap(), in_=qT.bitcast(f32))
                  nc.sync.dma_start(out=dbg_kT.ap(), in_=kT.bitcast(f32))
                  nc.sync.dma_start(
                      out=dbg_vA.ap(),
                      in_=vA.rearrange("p h t e -> p (h t e)").bitcast(f32))

              # ================= Phase 2+3: attention + out-proj =========
              with tc.tile_pool(name=f"att{rep}", bufs=1) as ap_, \
                   tc.tile_pool(name=f"psB{rep}", bufs=1, space="PSUM") as psB:
                  onorm = ap_.tile([128, NPAIR, S], f32r, name=f"onorm_{rep}")   # o_norm^T pair-stacked
                  wo_sb = ap_.tile([128, NPAIR, DIM], f32r, name=f"wosb_{rep}")
                  nc.sync.dma_start(
                      out=wo_sb, in_=wo_d.ap().rearrange("(p q) m -> q p m", q=128))

                  for p in ([] if skip_attn else range(NPAIR)):
                      h0, h1 = 2 * p, 2 * p + 1
                      for sq in range(NSQ):
                          sqs = slice(sq * 512, (sq + 1) * 512)
                          o_ps = [psB.tile([65, 512], f32, name=f"o{p}_{sq}_{j}_{rep}",
                                           tag=f"o_ps{j}", bufs=1)
                                  for j in range(2)]
                          for ttg in range(NT // 2):
                              sblk = [psB.tile([128, 2, 512], f32,
                                               name=f"s{p}{sq}{ttg}{j}_{rep}",
                                               tag="blk", bufs=3)
                                      for j in range(2)]
                              att = [ap_.tile([128, 2, 512], f32r,
                                              name=f"a{p}{sq}{ttg}{j}_{rep}",
                                              tag=f"att{j}", bufs=2)
                                     for j in range(2)]
                              for jj in range(2):
                                  tt = 2 * ttg + jj
                                  for j, hh in ((0, h0), (1, h1)):
                                      lo = j * 64
                                      nc.tensor.matmul(
                                          sblk[j][:, jj, :],
                                          kT[lo:lo + 64, p,
                                             tt * 128:(tt + 1) * 128],
                                          qT[lo:lo + 64, p, sqs],
                                          start=True, stop=True)
                              for j in range(2):
                                  nc.scalar.activation(att[j][:, :, :], sblk[j],
                                                       AF.Exp, scale=SCALE)
                              if debug_taps and p == 0 and sq == 0 and ttg == 0:
                                  for j in range(2):
                                      nc.sync.dma_start(
                                          out=dbg_att.ap()[:, j, :, :],
                                          in_=att[j].bitcast(f32))
                              for jj in range(2):
                                  tt = 2 * ttg + jj
                                  for j, hh in ((0, h0), (1, h1)):
                                      nc.tensor.matmul(
                                          o_ps[j],
                                          vA[:, hh, tt, :],
                                          att[j][:, jj, :],
                                          start=(tt == 0), stop=(tt == NT - 1))
                          # ---- normalize (all DVE math at base partition 0;
                          #      cross-partition placement via tensor_copy) ----
                          for j in range(2):
                              rrow = ap_.tile([1, 512], f32, name=f"r{p}{sq}{j}_{rep}",
                                              tag=f"rrow{j}", bufs=2)
                              nc.vector.tensor_copy(rrow, o_ps[j][64:65, :])
                              rbc = ap_.tile([64, 512], f32, tag=f"rbc{j}",
                                             bufs=2, name=f"rbc{p}{sq}{j}_{rep}")
                              nc.gpsimd.partition_broadcast(rbc, rrow)
                              rinv = ap_.tile([64, 512], f32, tag=f"rinv{j}",
                                              bufs=2, name=f"rinv{p}{sq}{j}_{rep}")
                              nc.vector.reciprocal(rinv, rbc)
                              if j == 0:
                                  nc.vector.tensor_tensor(
                                      out=onorm[0:64, p, sqs],
                                      in0=o_ps[0][0:64, :], in1=rinv,
                                      op=Alu.mult)
                              else:
                                  tmp1 = ap_.tile([64, 512], f32r, tag="ntmp",
                                                  bufs=2, name=f"nt{p}{sq}_{rep}")
                                  nc.vector.tensor_tensor(
                                      out=tmp1, in0=o_ps[1][0:64, :], in1=rinv,
                                      op=Alu.mult)
                                  nc.vector.tensor_copy(
                                      onorm[64:128, p, sqs], tmp1)

                  if debug_taps:
                      nc.sync.dma_start(out=dbg_on.ap(), in_=onorm.bitcast(f32))
                  # ---- out-projection: y[s,:] = sum_p onorm_p^T @ Wo_p ----
                  for st in ([] if skip_oproj else range(NT)):
                      # [128, 1024] = 2 banks, same slot size as "blk" tiles
                      ps = psB.tile([128, DIM], f32, name=f"ps_y{st}",
                                    tag="blk", bufs=3)
                      for p in range(NPAIR):
                          lhs = onorm[:, p, st * 128:(st + 1) * 128]
                          for nh in range(DIM // 512):
                              nc.tensor.matmul(
                                  ps[:, nh * 512:(nh + 1) * 512],
                                  lhs,
                                  wo_sb[:, p, nh * 512:(nh + 1) * 512],
                                  start=(p == 0), stop=(p == NPAIR - 1))
                      ysb = ap_.tile([128, DIM], f32, tag="ysb", bufs=2,
                                     name=f"ysb{st}_{rep}")
                      nc.vector.tensor_copy(ysb, ps)
                      nc.sync.dma_start(out=y_d.ap()[st * 128:(st + 1) * 128, :],
                                        in_=ysb)
    nc.compile()
    return nc


def _get_nc():
    if "nc" not in _CACHE:
        _CACHE["nc"] = _build()
    return _CACHE["nc"]


def make_in_maps(x, Wq, Wk, Wv, bq, bk, bv, Wo, bo):
    in_maps = []
    for c in range(NCORES):
        b, g = c // 2, c % 2
        hs = slice(g * HPC, (g + 1) * HPC)
        in_maps.append({
            "xT": np.ascontiguousarray(x[b].T),
            "wq": np.ascontiguousarray(
                Wq[hs].transpose(1, 0, 2).reshape(DIM, HPC * DH)),
            "wk": np.ascontiguousarray(
                Wk[hs].transpose(1, 0, 2).reshape(DIM, HPC * DH)),
            "wv": np.ascontiguousarray(
                Wv[hs].transpose(1, 0, 2).reshape(DIM, HPC * DH)),
            "bq": np.ascontiguousarray(bq[hs].reshape(NPAIR, 128).T),
            "bk": np.ascontiguousarray(bk[hs].reshape(NPAIR, 128).T),
            "wo": np.ascontiguousarray(Wo[g * 512:(g + 1) * 512, :]),
        })
    return in_maps


def combine(results, bv, Wo, bo):
    const = bv.reshape(DIM) @ Wo + bo          # [DIM]
    y = np.empty((B, S, DIM), dtype=np.float32)
    for b in range(B):
        y[b] = results[2 * b]["y"] + results[2 * b + 1]["y"] + const
    return y


def kernel(x, Wq, Wk, Wv, bq, bk, bv, Wo, bo):
    import time
    from concourse.bass_utils import run_bass_kernel_spmd
    x, Wq, Wk, Wv, bq, bk, bv, Wo, bo = [
        np.asarray(a, dtype=np.float32)
        for a in (x, Wq, Wk, Wv, bq, bk, bv, Wo, bo)]
    nc = _get_nc()
    in_maps = make_in_maps(x, Wq, Wk, Wv, bq, bk, bv, Wo, bo)
    last = None
    for attempt in range(3):
        try:
            res = run_bass_kernel_spmd(nc, in_maps,
                                       core_ids=list(range(NCORES)))
            return combine(res.results, bv, Wo, bo)
        except Exception as e:  # transient NRT_EXEC_UNIT_UNRECOVERABLE wedges
            last = e
            time.sleep(75)
    raise last

